# revision 11
# baseline (speedup 1.0000x reference)
"""Trainium2 Bass kernel for nn_Attention_45569603010584.

Per-node causal conv attention + FFN over (B=32, C=64, N=207, T=96).
Shards the flattened b*n = 6624 attention-batch dim across 8 cores
(828 each). Each core processes its bns in groups of G=5 (plus a
remainder group of 3), batching all shared-weight matmuls and
elementwise work across the group; only the inherently per-bn
attention matmuls run per bn.

Layout per group (tokens = G*96 columns):
  qk conv   : 2 matmuls (tap2 on x, taps0/1 on shifted copies)
  vT        : per-bn matmul  lhsT=x_bn[64c,96t], rhs=v_wT -> [96t,32h]
  attnT     : per-bn matmul  lhsT=K_bn[32,96], rhs=Q_bn -> [96k,96q]
  mask      : one matmul     lhsT=I96, rhs=(-1000*mask) accumulated
  exp       : ACT on [96, tokens]
  attn_outT : per-bn matmul  lhsT=E_bn[96k,96q], rhs=[vT|1] -> [96q,33]
              (col 32 = softmax denominator)
  normalize : DVE reciprocal + broadcast multiply
  transpose : per-bn PE transpose [96q,32h] -> [32h,96q]
  o-proj    : matmul + residual add;  FFN: 2 matmuls + relu
"""

import numpy as np

B, C, N, T = 32, 64, 207, 96
H = 32
NCORES = 8
BN = B * N              # 6624
BN_CORE = BN // NCORES  # 828
G = 5                   # bns per group
GROUPS = [G] * (BN_CORE // G) + ([BN_CORE % G] if BN_CORE % G else [])
TOK_CORE = BN_CORE * T  # 79488

_CACHE = {}


def _make_tile_context_cls():
    import concourse.mybir as mybir
    from concourse.tile import TileContext, ScopedClock

    class PatchedTileContext(TileContext):
        """The walrus build here rejects instructions carrying more than
        ~2 semaphore waits ("Too many sync wait commands"); TileContext's
        kernel-tail drain aggregates one wait per logical processor onto a
        single Drain. Split them one-per-nop instead."""

        def _split_excess_waits(self):
            """Walrus here allows very few sem waits per TPB instruction;
            move extras onto preceding same-engine nops."""
            nsplit = 0
            for f in self.nc.m.functions:
                for bb in f.blocks:
                    il = bb.instructions
                    out = []
                    for inst in il:
                        si = inst.sync_info
                        if si is not None and len(si.on_wait) > 1:
                            waits = list(si.on_wait)
                            for i, w in enumerate(waits[:-1]):
                                nop = mybir.InstNoOp(
                                    name=f"{inst.name}_wsplit{i}",
                                    engine=inst.engine)
                                nop.sync_info = mybir.SyncInfo(
                                    on_wait=[w], on_update=[])
                                out.append(nop)
                                nsplit += 1
                            inst.sync_info = mybir.SyncInfo(
                                on_wait=waits[-1:],
                                on_update=list(si.on_update))
                        out.append(inst)
                    il[:] = out
            return nsplit

        def _drain_and_barrier(self, tick_clock, wait_clock):
            carrier = self.nc.sync.nop()
            wait_clock.add_sem_waits(
                carrier.ins, ScopedClock({None: tick_clock.global_clock}))
            si = carrier.ins.sync_info
            waits = list(si.on_wait) if si is not None else []
            upd = list(si.on_update) if si is not None else []
            carrier.ins.sync_info = mybir.SyncInfo(on_wait=waits[:1],
                                                   on_update=upd)
            for i in range(1, len(waits)):
                n2 = self.nc.sync.nop()
                n2.ins.sync_info = mybir.SyncInfo(on_wait=waits[i:i + 1],
                                                  on_update=[])
            self.nc.sync.drain()
            self.nc.all_engine_barrier()
            assert self.sems is not None
            popped = self.nc._tile_sem_poison_stack.pop()
            assert popped is self._sem_poison
            self.nc.clear_and_free_semaphores(
                list(self.sems.allocated().values()))
            self.nc.all_engine_barrier()
            self._split_excess_waits()

    return PatchedTileContext


def _build_program(groups=None, tok_total=None, stages=99):
    import concourse.bass as bass
    import concourse.mybir as mybir
    from contextlib import ExitStack

    if groups is None:
        groups = GROUPS
    if tok_total is None:
        tok_total = TOK_CORE
    TOKT = tok_total

    TileContext = _make_tile_context_cls()
    FP = mybir.dt.float32
    nc = bass.Bass()

    xin = nc.dram_tensor("xin", [C, TOKT], FP, kind="ExternalInput")
    wqk2_d = nc.dram_tensor("wqk2", [C, 2 * H], FP, kind="ExternalInput")
    wqk01_d = nc.dram_tensor("wqk01", [2 * C, 2 * H], FP, kind="ExternalInput")
    vwt_d = nc.dram_tensor("vwt", [C, H], FP, kind="ExternalInput")
    owt_d = nc.dram_tensor("owt", [H, C], FP, kind="ExternalInput")
    ff1t_d = nc.dram_tensor("ff1t", [C, C], FP, kind="ExternalInput")
    ff2t_d = nc.dram_tensor("ff2t", [C, C], FP, kind="ExternalInput")
    b1_d = nc.dram_tensor("b1", [C, 1], FP, kind="ExternalInput")
    mneg_d = nc.dram_tensor("mneg", [T, G * T], FP, kind="ExternalInput")
    ident_d = nc.dram_tensor("ident", [T, T], FP, kind="ExternalInput")
    yout = nc.dram_tensor("yout", [C, TOKT], FP, kind="ExternalOutput")

    with TileContext(nc) as tc, ExitStack() as ctx:  # noqa: F841
        const = ctx.enter_context(tc.tile_pool(name="const", bufs=1))

        def load_const(dram, shape, tag):
            t = const.tile(shape, FP, tag=tag)
            nc.sync.dma_start(out=t[:], in_=dram[:])
            return t

        wqk2_sb = load_const(wqk2_d, [C, 2 * H], "wqk2")
        wqk01_sb = load_const(wqk01_d, [2 * C, 2 * H], "wqk01")
        vwt_sb = load_const(vwt_d, [C, H], "vwt")
        owt_sb = load_const(owt_d, [H, C], "owt")
        ff1t_sb = load_const(ff1t_d, [C, C], "ff1t")
        ff2t_sb = load_const(ff2t_d, [C, C], "ff2t")
        b1_sb = load_const(b1_d, [C, 1], "b1")
        mneg_sb = load_const(mneg_d, [T, G * T], "mneg")
        ident_sb = load_const(ident_d, [T, T], "ident")

        xp = ctx.enter_context(tc.tile_pool(name="xp", bufs=3))
        sp = ctx.enter_context(tc.tile_pool(name="sp", bufs=2))
        qkp = ctx.enter_context(tc.tile_pool(name="qkp", bufs=2))
        k0p = ctx.enter_context(tc.tile_pool(name="k0p", bufs=2))
        vtp = ctx.enter_context(tc.tile_pool(name="vtp", bufs=2))
        ep = ctx.enter_context(tc.tile_pool(name="ep", bufs=2))
        rxp = ctx.enter_context(tc.tile_pool(name="rxp", bufs=2))
        atnp = ctx.enter_context(tc.tile_pool(name="atnp", bufs=2))
        aop = ctx.enter_context(tc.tile_pool(name="aop", bufs=2))
        ofp = ctx.enter_context(tc.tile_pool(name="ofp", bufs=2))
        h1p = ctx.enter_context(tc.tile_pool(name="h1p", bufs=2))
        outp = ctx.enter_context(tc.tile_pool(name="outp", bufs=3))

        ps_qkv = ctx.enter_context(tc.tile_pool(name="ps_qkv", bufs=1, space="PSUM"))
        ps_vt = ctx.enter_context(tc.tile_pool(name="ps_vt", bufs=1, space="PSUM"))
        ps_at = ctx.enter_context(tc.tile_pool(name="ps_at", bufs=1, space="PSUM"))
        ps_o = ctx.enter_context(tc.tile_pool(name="ps_o", bufs=1, space="PSUM"))
        ps_ao = ctx.enter_context(tc.tile_pool(name="ps_ao", bufs=1, space="PSUM"))
        ps_of = ctx.enter_context(tc.tile_pool(name="ps_of", bufs=1, space="PSUM"))
        ps_h1 = ctx.enter_context(tc.tile_pool(name="ps_h1", bufs=1, space="PSUM"))
        ps_ff = ctx.enter_context(tc.tile_pool(name="ps_ff", bufs=1, space="PSUM"))

        col = 0
        for g in groups:
            tok = g * T

            x_t = xp.tile([C, G * T], FP, tag="x")
            nc.sync.dma_start(out=x_t[:, :tok], in_=xin[:, col:col + tok])

            # shifted copies for conv taps 1 and 0 (left causal pad)
            s_t = sp.tile([2 * C, G * T], FP, tag="s")
            nc.sync.dma_start(out=s_t[0:C, 1:tok], in_=x_t[:, 0:tok - 1])
            nc.sync.dma_start(out=s_t[C:2 * C, 2:tok], in_=x_t[:, 0:tok - 2])
            s3 = s_t.rearrange("p (n t) -> p n t", t=T)
            nc.gpsimd.memset(s3[0:C, 0:g, 0:1], 0.0)
            nc.gpsimd.memset(s3[C:2 * C, 0:g, 0:2], 0.0)

            # qk conv: [64 rows: q 0-31, k 32-63] x tokens
            p_qkv = ps_qkv.tile([2 * H, G * T], FP, tag="qkv")
            nc.tensor.matmul(p_qkv[:, :tok], wqk2_sb[:], x_t[:, :tok],
                             start=True, stop=False)
            nc.tensor.matmul(p_qkv[:, :tok], wqk01_sb[:], s_t[:, :tok],
                             start=False, stop=True)
            qk_sb = qkp.tile([2 * H, G * T], FP, tag="qk")
            nc.scalar.copy(out=qk_sb[:, :tok], in_=p_qkv[:, :tok])
            # K rows to base partition 0 (matmul operands need same base)
            k0_sb = k0p.tile([H, G * T], FP, tag="k0")
            nc.sync.dma_start(out=k0_sb[:, :tok], in_=qk_sb[H:2 * H, :tok])

            if stages < 2:
                nc.sync.dma_start(out=yout[:, col:col + tok], in_=qk_sb[0:C, :tok])
                col += tok
                continue

            # vT per bn: [96t, 32h] each
            p_vt = ps_vt.tile([T, G * H], FP, tag="vt")
            for j in range(g):
                nc.tensor.matmul(p_vt[:, j * H:(j + 1) * H],
                                 x_t[:, j * T:(j + 1) * T], vwt_sb[:],
                                 start=True, stop=True, skip_group_check=True)
            vt_sb = vtp.tile([T, G * (H + 1)], FP, tag="vt_sb")
            vt3 = vt_sb.rearrange("p (n c) -> p n c", c=H + 1)
            nc.vector.tensor_copy(
                vt3[:, 0:g, 0:H],
                p_vt.rearrange("p (n c) -> p n c", c=H)[:, 0:g, :])
            nc.gpsimd.memset(vt3[:, 0:g, H:H + 1], 1.0)

            if stages < 3:
                nc.sync.dma_start(out=yout[:, col:col + tok], in_=x_t[:, :tok])
                col += tok
                continue

            # attnT = K^T Q per bn, then -1000*mask accumulated
            p_at = ps_at.tile([T, G * T], FP, tag="at")
            nc.tensor.matmul(p_at[:, :tok], ident_sb[:], mneg_sb[:, :tok],
                             start=True, stop=False, skip_group_check=True)
            for j in range(g):
                nc.tensor.matmul(p_at[:, j * T:(j + 1) * T],
                                 k0_sb[:, j * T:(j + 1) * T],
                                 qk_sb[0:H, j * T:(j + 1) * T],
                                 start=False, stop=(j == g - 1),
                                 skip_group_check=True)
            e_sb = ep.tile([T, G * T], FP, tag="e")
            nc.scalar.activation(out=e_sb[:, :tok], in_=p_at[:, :tok],
                                 func=mybir.ActivationFunctionType.Exp)

            if stages < 4:
                nc.sync.dma_start(out=yout[:, col:col + tok], in_=e_sb[0:C, :tok])
                col += tok
                continue

            # attn_outT (+denominator col) per bn
            p_o = ps_o.tile([T, G * (H + 1)], FP, tag="o")
            for j in range(g):
                nc.tensor.matmul(p_o[:, j * (H + 1):(j + 1) * (H + 1)],
                                 e_sb[:, j * T:(j + 1) * T],
                                 vt3[:, j, :],
                                 start=True, stop=True, skip_group_check=True)
            o3 = p_o.rearrange("p (n c) -> p n c", c=H + 1)
            rx = rxp.tile([T, G], FP, tag="rx")
            rx3 = rx.rearrange("p (n c) -> p n c", c=1)
            nc.vector.reciprocal(out=rx3[:, 0:g, :], in_=o3[:, 0:g, H:H + 1])
            rx_b = bass.AP(tensor=rx.tensor, offset=rx.offset,
                           ap=[rx.ap[0], [rx.ap[1][0], g], [0, H]])
            atn_sb = atnp.tile([T, G * H], FP, tag="atn")
            atn3 = atn_sb.rearrange("p (n c) -> p n c", c=H)
            nc.vector.tensor_mul(atn3[:, 0:g, :], o3[:, 0:g, 0:H], rx_b)

            if stages < 5:
                nc.sync.dma_start(out=yout[:, col:col + tok], in_=x_t[:, :tok])
                col += tok
                continue

            # transpose each [96q,32h] -> [32h,96q]
            p_ao = ps_ao.tile([H, G * T], FP, tag="ao")
            for j in range(g):
                nc.tensor.transpose(p_ao[:, j * T:(j + 1) * T],
                                    atn_sb[:, j * H:(j + 1) * H],
                                    ident_sb[:])
            ao_sb = aop.tile([H, G * T], FP, tag="ao_sb")
            nc.scalar.copy(out=ao_sb[:, :tok], in_=p_ao[:, :tok])

            if stages < 6:
                nc.sync.dma_start(out=yout[:, col:col + tok], in_=x_t[:, :tok])
                col += tok
                continue

            # out_f = x + o_w @ attn_out
            p_of = ps_of.tile([C, G * T], FP, tag="of")
            nc.tensor.matmul(p_of[:, :tok], owt_sb[:], ao_sb[:, :tok],
                             start=True, stop=True)
            of_sb = ofp.tile([C, G * T], FP, tag="of_sb")
            nc.vector.tensor_add(of_sb[:, :tok], p_of[:, :tok], x_t[:, :tok])

            if stages < 7:
                nc.sync.dma_start(out=yout[:, col:col + tok], in_=of_sb[:, :tok])
                col += tok
                continue

            # FFN
            p_h1 = ps_h1.tile([C, G * T], FP, tag="h1")
            nc.tensor.matmul(p_h1[:, :tok], ff1t_sb[:], of_sb[:, :tok],
                             start=True, stop=True)
            h1_sb = h1p.tile([C, G * T], FP, tag="h1_sb")
            nc.scalar.activation(out=h1_sb[:, :tok], in_=p_h1[:, :tok],
                                 func=mybir.ActivationFunctionType.Relu,
                                 bias=b1_sb[:, 0:1], scale=1.0)
            p_ff = ps_ff.tile([C, G * T], FP, tag="ff")
            nc.tensor.matmul(p_ff[:, :tok], ff2t_sb[:], h1_sb[:, :tok],
                             start=True, stop=True)
            out_t = outp.tile([C, G * T], FP, tag="out")
            nc.vector.tensor_add(out_t[:, :tok], p_ff[:, :tok], of_sb[:, :tok])

            nc.sync.dma_start(out=yout[:, col:col + tok], in_=out_t[:, :tok])
            col += tok

    return nc


def _prep_consts(q_w, k_w, v_w, o_w, ff_w1, ff_b1, ff_w2):
    f = np.float32
    wqk2 = np.ascontiguousarray(
        np.concatenate([q_w[:, :, 2], k_w[:, :, 2]], 0).T, dtype=f)
    wqk01 = np.ascontiguousarray(np.concatenate([
        np.concatenate([q_w[:, :, 1], k_w[:, :, 1]], 0).T,
        np.concatenate([q_w[:, :, 0], k_w[:, :, 0]], 0).T], 0), dtype=f)
    vwt = np.ascontiguousarray(v_w.T, dtype=f)
    owt = np.ascontiguousarray(o_w.T, dtype=f)
    ff1t = np.ascontiguousarray(ff_w1.T, dtype=f)
    ff2t = np.ascontiguousarray(ff_w2.T, dtype=f)
    b1 = np.ascontiguousarray(ff_b1.reshape(C, 1), dtype=f)
    mneg1 = np.where(np.arange(T)[:, None] > np.arange(T)[None, :],
                     f(-1000.0), f(0.0)).astype(f)
    mneg = np.ascontiguousarray(np.tile(mneg1, (1, G)))
    ident = np.eye(T, dtype=f)
    return dict(wqk2=wqk2, wqk01=wqk01, vwt=vwt, owt=owt, ff1t=ff1t,
                ff2t=ff2t, b1=b1, mneg=mneg, ident=ident)


def kernel(x, q_w, k_w, v_w, o_w, ff_w1, ff_b1, ff_w2, ff_b2):
    from concourse.bass_utils import run_bass_kernel_spmd

    if "nc" not in _CACHE:
        _CACHE["nc"] = _build_program()
    nc = _CACHE["nc"]

    consts = _prep_consts(q_w, k_w, v_w, o_w, ff_w1, ff_b1, ff_w2)
    xt = np.ascontiguousarray(
        x.transpose(1, 0, 2, 3).reshape(C, BN, T), dtype=np.float32)

    in_maps = []
    for i in range(NCORES):
        xc = np.ascontiguousarray(
            xt[:, i * BN_CORE:(i + 1) * BN_CORE, :].reshape(C, TOK_CORE))
        in_maps.append({"xin": xc, **consts})

    try:
        res = run_bass_kernel_spmd(nc, in_maps, list(range(NCORES)))
    except Exception:
        # a previously wedged device typically clears on retry
        res = run_bass_kernel_spmd(nc, in_maps, list(range(NCORES)))

    out = np.empty((C, BN, T), np.float32)
    for i in range(NCORES):
        out[:, i * BN_CORE:(i + 1) * BN_CORE, :] = \
            res.results[i]["yout"].reshape(C, BN_CORE, T)
    out = out.reshape(C, B, N, T).transpose(1, 0, 2, 3)
    # ff_b2 is added on host (it is all-zeros in this problem's inputs)
    out = out + np.asarray(ff_b2, np.float32)[None, :, None, None]
    return np.ascontiguousarray(out)


# revision 15
# speedup vs baseline: 1.0434x; 1.0434x over previous
"""Trainium2 Bass kernel for nn_Attention_45569603010584.

Per-node causal conv attention + FFN over (B=32, C=64, N=207, T=96).
Shards the flattened b*n = 6624 attention-batch dim across 8 cores
(828 each). Each core processes its bns in groups of G=5 (plus a
remainder group of 3), batching all shared-weight matmuls and
elementwise work across the group; only the inherently per-bn
attention matmuls run per bn.

Layout per group (tokens = G*96 columns):
  qk conv   : 2 matmuls (tap2 on x, taps0/1 on shifted copies)
  vT        : per-bn matmul  lhsT=x_bn[64c,96t], rhs=v_wT -> [96t,32h]
  attnT     : per-bn matmul  lhsT=K_bn[32,96], rhs=Q_bn -> [96k,96q]
  mask      : one matmul     lhsT=I96, rhs=(-1000*mask) accumulated
  exp       : ACT on [96, tokens]
  attn_outT : per-bn matmul  lhsT=E_bn[96k,96q], rhs=[vT|1] -> [96q,33]
              (col 32 = softmax denominator)
  normalize : DVE reciprocal + broadcast multiply
  transpose : per-bn PE transpose [96q,32h] -> [32h,96q]
  o-proj    : matmul + residual add;  FFN: 2 matmuls + relu
"""

import numpy as np

B, C, N, T = 32, 64, 207, 96
H = 32
NCORES = 8
BN = B * N              # 6624
BN_CORE = BN // NCORES  # 828
G = 5                   # bns per group
GROUPS = [G] * (BN_CORE // G) + ([BN_CORE % G] if BN_CORE % G else [])
TOK_CORE = BN_CORE * T  # 79488

_CACHE = {}


def _make_tile_context_cls():
    import concourse.mybir as mybir
    from concourse.tile import TileContext, ScopedClock

    class PatchedTileContext(TileContext):
        """The walrus build here rejects instructions carrying more than
        ~2 semaphore waits ("Too many sync wait commands"); TileContext's
        kernel-tail drain aggregates one wait per logical processor onto a
        single Drain. Split them one-per-nop instead."""

        def _split_excess_waits(self):
            """Walrus here allows very few sem waits per TPB instruction;
            move extras onto preceding same-engine nops."""
            nsplit = 0
            for f in self.nc.m.functions:
                for bb in f.blocks:
                    il = bb.instructions
                    out = []
                    for inst in il:
                        si = inst.sync_info
                        if si is not None and len(si.on_wait) > 1:
                            waits = list(si.on_wait)
                            for i, w in enumerate(waits[:-1]):
                                nop = mybir.InstNoOp(
                                    name=f"{inst.name}_wsplit{i}",
                                    engine=inst.engine)
                                nop.sync_info = mybir.SyncInfo(
                                    on_wait=[w], on_update=[])
                                out.append(nop)
                                nsplit += 1
                            inst.sync_info = mybir.SyncInfo(
                                on_wait=waits[-1:],
                                on_update=list(si.on_update))
                        out.append(inst)
                    il[:] = out
            return nsplit

        def _drain_and_barrier(self, tick_clock, wait_clock):
            carrier = self.nc.sync.nop()
            wait_clock.add_sem_waits(
                carrier.ins, ScopedClock({None: tick_clock.global_clock}))
            si = carrier.ins.sync_info
            waits = list(si.on_wait) if si is not None else []
            upd = list(si.on_update) if si is not None else []
            carrier.ins.sync_info = mybir.SyncInfo(on_wait=waits[:1],
                                                   on_update=upd)
            for i in range(1, len(waits)):
                n2 = self.nc.sync.nop()
                n2.ins.sync_info = mybir.SyncInfo(on_wait=waits[i:i + 1],
                                                  on_update=[])
            self.nc.sync.drain()
            self.nc.all_engine_barrier()
            assert self.sems is not None
            popped = self.nc._tile_sem_poison_stack.pop()
            assert popped is self._sem_poison
            self.nc.clear_and_free_semaphores(
                list(self.sems.allocated().values()))
            self.nc.all_engine_barrier()
            self._split_excess_waits()

    return PatchedTileContext


def _build_program(groups=None, tok_total=None, stages=99):
    import concourse.bass as bass
    import concourse.mybir as mybir
    from contextlib import ExitStack

    if groups is None:
        groups = GROUPS
    if tok_total is None:
        tok_total = TOK_CORE
    TOKT = tok_total

    TileContext = _make_tile_context_cls()
    FP = mybir.dt.float32
    FR = mybir.dt.float32r
    nc = bass.Bass()

    xin = nc.dram_tensor("xin", [C, TOKT], FR, kind="ExternalInput")
    wqk2_d = nc.dram_tensor("wqk2", [C, 2 * H], FR, kind="ExternalInput")
    wqk01_d = nc.dram_tensor("wqk01", [2 * C, 2 * H], FR, kind="ExternalInput")
    vwt_d = nc.dram_tensor("vwt", [C, H], FR, kind="ExternalInput")
    owt_d = nc.dram_tensor("owt", [H, C], FR, kind="ExternalInput")
    ff1t_d = nc.dram_tensor("ff1t", [C, C], FR, kind="ExternalInput")
    ff2t_d = nc.dram_tensor("ff2t", [C, C], FR, kind="ExternalInput")
    b1_d = nc.dram_tensor("b1", [C, 1], FP, kind="ExternalInput")
    mneg_d = nc.dram_tensor("mneg", [T, G * T], FR, kind="ExternalInput")
    ident_d = nc.dram_tensor("ident", [T, T], FR, kind="ExternalInput")
    yout = nc.dram_tensor("yout", [C, TOKT], FP, kind="ExternalOutput")

    with TileContext(nc) as tc, ExitStack() as ctx:  # noqa: F841
        const = ctx.enter_context(tc.tile_pool(name="const", bufs=1))

        def load_const(dram, shape, tag, dt=None):
            t = const.tile(shape, dt or FR, tag=tag)
            nc.sync.dma_start(out=t[:], in_=dram[:])
            return t

        wqk2_sb = load_const(wqk2_d, [C, 2 * H], "wqk2")
        wqk01_sb = load_const(wqk01_d, [2 * C, 2 * H], "wqk01")
        vwt_sb = load_const(vwt_d, [C, H], "vwt")
        owt_sb = load_const(owt_d, [H, C], "owt")
        ff1t_sb = load_const(ff1t_d, [C, C], "ff1t")
        ff2t_sb = load_const(ff2t_d, [C, C], "ff2t")
        b1_sb = load_const(b1_d, [C, 1], "b1", dt=FP)
        mneg_sb = load_const(mneg_d, [T, G * T], "mneg")
        ident_sb = load_const(ident_d, [T, T], "ident")

        xp = ctx.enter_context(tc.tile_pool(name="xp", bufs=3))
        sp = ctx.enter_context(tc.tile_pool(name="sp", bufs=2))
        qkp = ctx.enter_context(tc.tile_pool(name="qkp", bufs=2))
        k0p = ctx.enter_context(tc.tile_pool(name="k0p", bufs=2))
        vtp = ctx.enter_context(tc.tile_pool(name="vtp", bufs=2))
        ep = ctx.enter_context(tc.tile_pool(name="ep", bufs=2))
        rxp = ctx.enter_context(tc.tile_pool(name="rxp", bufs=2))
        atnp = ctx.enter_context(tc.tile_pool(name="atnp", bufs=2))
        aop = ctx.enter_context(tc.tile_pool(name="aop", bufs=2))
        ofp = ctx.enter_context(tc.tile_pool(name="ofp", bufs=2))
        h1p = ctx.enter_context(tc.tile_pool(name="h1p", bufs=2))
        outp = ctx.enter_context(tc.tile_pool(name="outp", bufs=3))

        ps_qkv = ctx.enter_context(tc.tile_pool(name="ps_qkv", bufs=1, space="PSUM"))
        ps_vt = ctx.enter_context(tc.tile_pool(name="ps_vt", bufs=1, space="PSUM"))
        ps_at = ctx.enter_context(tc.tile_pool(name="ps_at", bufs=1, space="PSUM"))
        ps_o = ctx.enter_context(tc.tile_pool(name="ps_o", bufs=1, space="PSUM"))
        ps_ao = ctx.enter_context(tc.tile_pool(name="ps_ao", bufs=1, space="PSUM"))
        ps_of = ctx.enter_context(tc.tile_pool(name="ps_of", bufs=1, space="PSUM"))
        ps_h1 = ctx.enter_context(tc.tile_pool(name="ps_h1", bufs=1, space="PSUM"))
        ps_ff = ctx.enter_context(tc.tile_pool(name="ps_ff", bufs=1, space="PSUM"))

        col = 0
        for g in groups:
            tok = g * T

            x_t = xp.tile([C, G * T], FR, tag="x")
            nc.sync.dma_start(out=x_t[:, :tok], in_=xin[:, col:col + tok])

            # shifted copies for conv taps 1 and 0 (left causal pad)
            s_t = sp.tile([2 * C, G * T], FR, tag="s")
            nc.sync.dma_start(out=s_t[0:C, 1:tok], in_=x_t[:, 0:tok - 1])
            nc.sync.dma_start(out=s_t[C:2 * C, 2:tok], in_=x_t[:, 0:tok - 2])
            s3 = s_t.bitcast(FP).rearrange("p (n t) -> p n t", t=T)
            nc.gpsimd.memset(s3[0:C, 0:g, 0:1], 0.0)
            nc.gpsimd.memset(s3[C:2 * C, 0:g, 0:2], 0.0)

            # qk conv: [64 rows: q 0-31, k 32-63] x tokens
            p_qkv = ps_qkv.tile([2 * H, G * T], FP, tag="qkv")
            nc.tensor.matmul(p_qkv[:, :tok], wqk2_sb[:], x_t[:, :tok],
                             start=True, stop=False)
            nc.tensor.matmul(p_qkv[:, :tok], wqk01_sb[:], s_t[:, :tok],
                             start=False, stop=True)
            qk_sb = qkp.tile([2 * H, G * T], FR, tag="qk")
            nc.scalar.copy(out=qk_sb[:, :tok], in_=p_qkv[:, :tok])
            # K rows to base partition 0 (matmul operands need same base)
            k0_sb = k0p.tile([H, G * T], FR, tag="k0")
            nc.sync.dma_start(out=k0_sb[:, :tok], in_=qk_sb[H:2 * H, :tok])

            if stages < 2:
                nc.sync.dma_start(out=yout[:, col:col + tok], in_=qk_sb[0:C, :tok])
                col += tok
                continue

            # vT per bn: [96t, 32h] each
            p_vt = ps_vt.tile([T, G * H], FP, tag="vt")
            for j in range(g):
                nc.tensor.matmul(p_vt[:, j * H:(j + 1) * H],
                                 x_t[:, j * T:(j + 1) * T], vwt_sb[:],
                                 start=True, stop=True, skip_group_check=True)
            vt_sb = vtp.tile([T, G * (H + 1)], FR, tag="vt_sb")
            vt3 = vt_sb.rearrange("p (n c) -> p n c", c=H + 1)
            nc.vector.tensor_copy(
                vt3[:, 0:g, 0:H],
                p_vt.rearrange("p (n c) -> p n c", c=H)[:, 0:g, :])
            nc.gpsimd.memset(
                vt_sb.bitcast(FP).rearrange("p (n c) -> p n c", c=H + 1)
                [:, 0:g, H:H + 1], 1.0)

            if stages < 3:
                nc.sync.dma_start(out=yout[:, col:col + tok], in_=x_t[:, :tok])
                col += tok
                continue

            # attnT = K^T Q per bn, then -1000*mask accumulated
            p_at = ps_at.tile([T, G * T], FP, tag="at")
            nc.tensor.matmul(p_at[:, :tok], ident_sb[:], mneg_sb[:, :tok],
                             start=True, stop=False, skip_group_check=True)
            for j in range(g):
                nc.tensor.matmul(p_at[:, j * T:(j + 1) * T],
                                 k0_sb[:, j * T:(j + 1) * T],
                                 qk_sb[0:H, j * T:(j + 1) * T],
                                 start=False, stop=(j == g - 1),
                                 skip_group_check=True)
            e_sb = ep.tile([T, G * T], FR, tag="e")
            nc.scalar.activation(out=e_sb[:, :tok], in_=p_at[:, :tok],
                                 func=mybir.ActivationFunctionType.Exp)

            if stages < 4:
                nc.sync.dma_start(out=yout[:, col:col + tok], in_=e_sb[0:C, :tok])
                col += tok
                continue

            # attn_outT (+denominator col) per bn
            p_o = ps_o.tile([T, G * (H + 1)], FP, tag="o")
            for j in range(g):
                nc.tensor.matmul(p_o[:, j * (H + 1):(j + 1) * (H + 1)],
                                 e_sb[:, j * T:(j + 1) * T].bitcast(FP),
                                 vt3[:, j, :].bitcast(FP),
                                 start=True, stop=True, skip_group_check=True)
            o3 = p_o.rearrange("p (n c) -> p n c", c=H + 1)
            rx = rxp.tile([T, G], FP, tag="rx")
            rx3 = rx.rearrange("p (n c) -> p n c", c=1)
            nc.vector.reciprocal(out=rx3[:, 0:g, :], in_=o3[:, 0:g, H:H + 1])
            rx_b = bass.AP(tensor=rx.tensor, offset=rx.offset,
                           ap=[rx.ap[0], [rx.ap[1][0], g], [0, H]])
            atn_sb = atnp.tile([T, G * H], FR, tag="atn")
            atn3 = atn_sb.rearrange("p (n c) -> p n c", c=H)
            nc.vector.tensor_mul(atn3[:, 0:g, :], o3[:, 0:g, 0:H], rx_b)

            if stages < 5:
                nc.sync.dma_start(out=yout[:, col:col + tok], in_=x_t[:, :tok])
                col += tok
                continue

            # transpose each [96q,32h] -> [32h,96q]
            p_ao = ps_ao.tile([H, G * T], FR, tag="ao")
            for j in range(g):
                nc.tensor.transpose(p_ao[:, j * T:(j + 1) * T],
                                    atn_sb[:, j * H:(j + 1) * H],
                                    ident_sb[:])
            ao_sb = aop.tile([H, G * T], FR, tag="ao_sb")
            nc.scalar.copy(out=ao_sb[:, :tok], in_=p_ao[:, :tok])

            if stages < 6:
                nc.sync.dma_start(out=yout[:, col:col + tok], in_=x_t[:, :tok])
                col += tok
                continue

            # out_f = x + o_w @ attn_out
            p_of = ps_of.tile([C, G * T], FP, tag="of")
            nc.tensor.matmul(p_of[:, :tok], owt_sb[:], ao_sb[:, :tok],
                             start=True, stop=True)
            of_sb = ofp.tile([C, G * T], FR, tag="of_sb")
            nc.vector.tensor_add(of_sb[:, :tok], p_of[:, :tok],
                                 x_t[:, :tok].bitcast(FP))

            if stages < 7:
                nc.sync.dma_start(out=yout[:, col:col + tok], in_=of_sb[:, :tok])
                col += tok
                continue

            # FFN
            p_h1 = ps_h1.tile([C, G * T], FP, tag="h1")
            nc.tensor.matmul(p_h1[:, :tok], ff1t_sb[:], of_sb[:, :tok],
                             start=True, stop=True)
            h1_sb = h1p.tile([C, G * T], FR, tag="h1_sb")
            nc.scalar.activation(out=h1_sb[:, :tok], in_=p_h1[:, :tok],
                                 func=mybir.ActivationFunctionType.Relu,
                                 bias=b1_sb[:, 0:1], scale=1.0)
            p_ff = ps_ff.tile([C, G * T], FP, tag="ff")
            nc.tensor.matmul(p_ff[:, :tok], ff2t_sb[:], h1_sb[:, :tok],
                             start=True, stop=True)
            out_t = outp.tile([C, G * T], FP, tag="out")
            nc.vector.tensor_add(out_t[:, :tok], p_ff[:, :tok],
                                 of_sb[:, :tok].bitcast(FP))

            nc.sync.dma_start(out=yout[:, col:col + tok], in_=out_t[:, :tok])
            col += tok

    return nc


def _prep_consts(q_w, k_w, v_w, o_w, ff_w1, ff_b1, ff_w2):
    f = np.float32
    wqk2 = np.ascontiguousarray(
        np.concatenate([q_w[:, :, 2], k_w[:, :, 2]], 0).T, dtype=f)
    wqk01 = np.ascontiguousarray(np.concatenate([
        np.concatenate([q_w[:, :, 1], k_w[:, :, 1]], 0).T,
        np.concatenate([q_w[:, :, 0], k_w[:, :, 0]], 0).T], 0), dtype=f)
    vwt = np.ascontiguousarray(v_w.T, dtype=f)
    owt = np.ascontiguousarray(o_w.T, dtype=f)
    ff1t = np.ascontiguousarray(ff_w1.T, dtype=f)
    ff2t = np.ascontiguousarray(ff_w2.T, dtype=f)
    b1 = np.ascontiguousarray(ff_b1.reshape(C, 1), dtype=f)
    mneg1 = np.where(np.arange(T)[:, None] > np.arange(T)[None, :],
                     f(-1000.0), f(0.0)).astype(f)
    mneg = np.ascontiguousarray(np.tile(mneg1, (1, G)))
    ident = np.eye(T, dtype=f)
    return dict(wqk2=wqk2, wqk01=wqk01, vwt=vwt, owt=owt, ff1t=ff1t,
                ff2t=ff2t, b1=b1, mneg=mneg, ident=ident)


def kernel(x, q_w, k_w, v_w, o_w, ff_w1, ff_b1, ff_w2, ff_b2):
    from concourse.bass_utils import run_bass_kernel_spmd

    if "nc" not in _CACHE:
        _CACHE["nc"] = _build_program()
    nc = _CACHE["nc"]

    consts = _prep_consts(q_w, k_w, v_w, o_w, ff_w1, ff_b1, ff_w2)
    xt = np.ascontiguousarray(
        x.transpose(1, 0, 2, 3).reshape(C, BN, T), dtype=np.float32)

    in_maps = []
    for i in range(NCORES):
        xc = np.ascontiguousarray(
            xt[:, i * BN_CORE:(i + 1) * BN_CORE, :].reshape(C, TOK_CORE))
        in_maps.append({"xin": xc, **consts})

    try:
        res = run_bass_kernel_spmd(nc, in_maps, list(range(NCORES)))
    except Exception:
        # a previously wedged device typically clears on retry
        res = run_bass_kernel_spmd(nc, in_maps, list(range(NCORES)))

    out = np.empty((C, BN, T), np.float32)
    for i in range(NCORES):
        out[:, i * BN_CORE:(i + 1) * BN_CORE, :] = \
            res.results[i]["yout"].reshape(C, BN_CORE, T)
    out = out.reshape(C, B, N, T).transpose(1, 0, 2, 3)
    # ff_b2 is added on host (it is all-zeros in this problem's inputs)
    out = out + np.asarray(ff_b2, np.float32)[None, :, None, None]
    return np.ascontiguousarray(out)
